# revision 24
# baseline (speedup 1.0000x reference)
"""Form-A GNN attention kernel (v19).

out = (0.1*(deg/Z)*attm + 0.9*adj) @ in2, attm = adj*exp(lrelu(e1+e2)-8).

Per core: 1024 rows, two 512-row PSUM row-groups (rg).
- attm/adjT are the matmul STATIONARY operands ([128j,128row] slices); the
  moving operand is in2 augmented with a ones column (257 cols), so Z and
  deg fall out free as accumulator column 256. 1024 MMs x 257 cols.
- o1 (attention) matmuls run one 8-jc group BEHIND o2 (adjacency) matmuls.
- v19:
  * adjacency ships fp8e4 (binary -> exact): halves adj DMA to 8.4MB.
    o2 matmuls run fp8 stationary x bf16 moving; masks read fp8 directly.
  * all 16-bit elementwise tensors are bf16 (STT 2-port mode wants bf16
    sources); e1/u1 rounding is row-constant so it cancels in softmax.
  * DVE path per jo: tsm (u2*v2) + scalar_tensor_tensor (u1*v1 max t2);
    one batched mask-mul per group. nact=3 tiles take the ScalarE
    Prelu+Exp path with gpsimd mask-muls.
  * dependency tracking is tile-granular -> every piecewise-consumed
    buffer is its own tile: in2aug is 8+1 per-group tiles (first group
    split 2/6), first adj group split in half, e1b/u16 split per rg.
  * PSUM: 8 persistent single-bank tiles; o1/o2 swap bank sets per rg so
    the next rg's o2 stream reuses banks freed by the ScalarE side of
    the combine, not the Vector side.
  * combine (direct PSUM reads, no evacuation copies) is emitted in
    batched phases [zr rz c1]x4 -> [t]x4 -> [outt]x4 under
    tc.high_priority(); o1's stop group is emitted c-major so bank c0
    frees ~24 MMs before the group ends.
  * output f16, host upcasts; out DMAs alternate the two HW queues.
- Host prep: e1=input1@a1, e2=input2@a2, exp tables, dtype casts, adj
  transpose (same class of preprocessing as the original baseline).
"""

import numpy as np
import ml_dtypes
from contextlib import ExitStack

import concourse.bass as bass
import concourse.bacc as bacc
import concourse.tile as tile
from concourse import mybir
from concourse.bass_utils import run_bass_kernel_spmd

F32 = mybir.dt.float32
F16 = mybir.dt.float16
BF16 = mybir.dt.bfloat16
F8 = mybir.dt.float8e4

N_CORES = 8
N, M, D = 8192, 8192, 256
GAMMA = 0.1
P = 128
RB = 512
JC = M // P          # 64
JPG = 8
SHIFT = 4.0
AOP = mybir.AluOpType
AF = mybir.ActivationFunctionType

N_ACT = 4            # ACT-path tiles per group
ONE9 = float(np.float32(ml_dtypes.bfloat16(0.9)))   # ones-column scale


def build_kernel(nc, tc, ctx, rows):
    KR = rows // RB
    NC4 = RB // P
    NG = JC // JPG

    adjA_d = nc.dram_tensor("adjA", [KR, NG, P, N_ACT, RB], F8,
                            kind="ExternalInput").ap()
    adjB_d = nc.dram_tensor("adjB", [KR, NG, P, JPG - N_ACT, RB], BF16,
                            kind="ExternalInput").ap()
    in2aug_d = nc.dram_tensor("in2aug", [P, JC, D + 1], BF16,
                              kind="ExternalInput").ap()
    e1b_d = nc.dram_tensor("e1b", [P, KR, RB], BF16, kind="ExternalInput").ap()
    u16_d = nc.dram_tensor("u16", [P, KR, 2, RB], BF16,
                           kind="ExternalInput").ap()
    vcol_d = nc.dram_tensor("vcol", [P, 3, JC], F32, kind="ExternalInput").ap()
    out_d = nc.dram_tensor("outs", [rows, D], F16, kind="ExternalOutput").ap()

    const_pool = ctx.enter_context(tc.tile_pool(name="const", bufs=1))
    adj_pool = ctx.enter_context(tc.tile_pool(name="adj", bufs=7))
    work_pool = ctx.enter_context(tc.tile_pool(name="work", bufs=3))
    quad_pool = ctx.enter_context(tc.tile_pool(name="quad", bufs=4))
    tail_pool = ctx.enter_context(tc.tile_pool(name="tail", bufs=2))
    out_pool = ctx.enter_context(tc.tile_pool(name="outp", bufs=3))
    ps_pool = ctx.enter_context(tc.tile_pool(name="ps", bufs=1, space="PSUM"))

    adjg = {}

    def load_group(rg, g, eng=None):
        if g >= NG:
            return
        ga = adj_pool.tile([P, N_ACT, RB], F8, tag="ga",
                           name=f"ga_{rg}_{g}", bufs=10)
        gb = adj_pool.tile([P, JPG - N_ACT, RB], BF16, tag="gb",
                           name=f"gb_{rg}_{g}", bufs=10)
        adjg[(rg, g)] = (ga, gb)
        idx = rg * NG + g
        ea = eng or (nc.sync, nc.scalar)[idx % 2]
        eb = eng or (nc.sync, nc.scalar)[(idx + 1) % 2]
        ea.dma_start(out=ga[:], in_=adjA_d[rg, g])
        eb.dma_start(out=gb[:], in_=adjB_d[rg, g])

    # ---- persistent tiles; piecewise-consumed buffers get their own tile
    e1b = [const_pool.tile([P, RB], BF16, tag=f"e1b{r}", name=f"e1b{r}")
           for r in range(KR)]
    u16 = [const_pool.tile([P, 2, RB], BF16, tag=f"u16_{r}", name=f"u16_{r}")
           for r in range(KR)]
    vcol = const_pool.tile([P, 3, JC], F32, tag="vcol")
    negc = const_pool.tile([P, 1], F32, tag="negc")
    nc.vector.memset(negc[:], -2.0 * SHIFT)
    epsc = const_pool.tile([P, 1], F32, tag="epsc")
    nc.vector.memset(epsc[:], 1e-30)

    # in2aug: first group split 2/6 so the first matmul gates on ~132KB
    in2t = {}
    in2t[(0, 'a')] = const_pool.tile([P, 2, D + 1], BF16, tag="in2t0a",
                                     name="in2t0a")
    in2t[(0, 'b')] = const_pool.tile([P, JPG - 2, D + 1], BF16, tag="in2t0b",
                                     name="in2t0b")
    for g in range(1, NG):
        in2t[g] = const_pool.tile([P, JPG, D + 1], BF16, tag=f"in2t{g}",
                                  name=f"in2t{g}")

    def in2s(jc):
        g, jo = jc // JPG, jc % JPG
        if g == 0:
            return in2t[(0, 'a')][:, jo, :] if jo < 2 \
                else in2t[(0, 'b')][:, jo - 2, :]
        return in2t[g][:, jo, :]

    ga00 = adj_pool.tile([P, N_ACT, RB], F8, tag="ga", name="ga_0_0",
                         bufs=10)
    gb00 = adj_pool.tile([P, JPG - N_ACT, RB], BF16, tag="gb", name="gb_0_0",
                         bufs=10)
    adjg[(0, 0)] = (ga00, gb00)
    nc.sync.dma_start(out=ga00[:], in_=adjA_d[0, 0])
    nc.sync.dma_start(out=in2t[(0, 'a')][:], in_=in2aug_d[:, 0:2, :])
    nc.scalar.dma_start(out=vcol[:], in_=vcol_d)
    nc.scalar.dma_start(out=e1b[0][:], in_=e1b_d[:, 0, :])
    nc.scalar.dma_start(out=gb00[:], in_=adjB_d[0, 0])
    nc.scalar.dma_start(out=u16[0][:], in_=u16_d[:, 0, :, :])
    nc.sync.dma_start(out=in2t[(0, 'b')][:], in_=in2aug_d[:, 2:8, :])
    load_group(0, 1)
    nc.sync.dma_start(out=in2t[1][:], in_=in2aug_d[:, 8:16, :])
    load_group(0, 2)
    load_group(0, 3)
    nc.sync.dma_start(out=in2t[2][:], in_=in2aug_d[:, 16:24, :])
    load_group(0, 4)
    load_group(0, 5)
    nc.scalar.dma_start(out=in2t[3][:], in_=in2aug_d[:, 24:32, :])
    nc.sync.dma_start(out=in2t[4][:], in_=in2aug_d[:, 32:40, :])
    for r in range(1, KR):
        nc.scalar.dma_start(out=e1b[r][:], in_=e1b_d[:, r, :])
        nc.scalar.dma_start(out=u16[r][:], in_=u16_d[:, r, :, :])
    nc.scalar.dma_start(out=in2t[5][:], in_=in2aug_d[:, 40:48, :])
    nc.sync.dma_start(out=in2t[6][:], in_=in2aug_d[:, 48:56, :])
    nc.scalar.dma_start(out=in2t[7][:], in_=in2aug_d[:, 56:64, :])

    e2s = vcol[:, 0, :]
    v1 = vcol[:, 1, :]
    v2 = vcol[:, 2, :]

    # 8 persistent single-bank PSUM tiles; role-swap per rg parity
    bank = [ps_pool.tile([P, 512], F32, tag=f"bank{i}", name=f"bank{i}")
            for i in range(8)]

    def elementwise(rg, jg):
        ga, gb = adjg.pop((rg, jg))

        def ags(jo):
            return ga[:, jo, :] if jo < N_ACT else gb[:, jo - N_ACT, :]

        # last two groups of each rg lean harder on ScalarE/GpSimd so
        # Vector is drained ahead of the rg-boundary stall cluster
        nact = 5 if jg >= NG - 4 else N_ACT
        ndve = JPG - nact
        attq = quad_pool.tile([P, ndve, RB], BF16, tag="attq",
                              name=f"attq_{rg}_{jg}")
        t2q = quad_pool.tile([P, ndve, RB], BF16, tag="t2q",
                             name=f"t2q_{rg}_{jg}")
        mq = quad_pool.tile([P, ndve, RB], BF16, tag="mq",
                            name=f"mq_{rg}_{jg}")
        atta = []
        for jo in range(JPG):
            jc = jg * JPG + jo
            if jo < nact:  # ACT path; mask-mul on gpsimd
                lr = work_pool.tile([P, RB], BF16, tag="lr", bufs=4)
                nc.scalar.activation(lr[:], e1b[rg][:], AF.Prelu,
                                     bias=e2s[:, jc:jc + 1], scale=1.0,
                                     alpha=0.2)
                ex = work_pool.tile([P, RB], BF16, tag="ex", bufs=6)
                nc.scalar.activation(ex[:], lr[:], AF.Exp, bias=negc[:])
                am = work_pool.tile([P, RB], BF16, tag="attm", bufs=12,
                                    name=f"attm_{rg}_{jg}_{jo}")
                nc.gpsimd.tensor_mul(am[:], ex[:], ags(jo))
                atta.append(am)
            else:  # DVE path: t1=u1*v1, t2=u2*v2, then batched max
                q = jo - nact
                nc.vector.tensor_scalar_mul(mq[:, q, :], u16[rg][:, 0, :],
                                            v1[:, jc:jc + 1])
                nc.vector.tensor_scalar_mul(t2q[:, q, :], u16[rg][:, 1, :],
                                            v2[:, jc:jc + 1])
        nc.vector.tensor_tensor(out=mq[:], in0=mq[:], in1=t2q[:],
                                op=AOP.max)
        nc.vector.tensor_mul(attq[:], mq[:], gb[:, nact - N_ACT:, :])
        stats = [(atta[jo] if jo < nact else attq[:, jo - nact, :])
                 for jo in range(JPG)]
        return stats, ags

    stash = None
    for rg in range(KR):
        if rg % 2 == 0:
            o1c, o2c = bank[0:NC4], bank[NC4:2 * NC4]
        else:
            o1c, o2c = bank[NC4:2 * NC4], bank[0:NC4]
        prev = None

        def emit_o1(stats, jg_of, stop_g, cs=None):
            order = ([(c, jo) for c in (cs or range(NC4))
                      for jo in range(JPG)]
                     if stop_g else
                     [(c, jo) for jo in range(JPG) for c in range(NC4)])
            for c, jo in order:
                nc.tensor.matmul(o1c[c][:, 0:D + 1],
                                 stats[jo][:, c * P:(c + 1) * P],
                                 in2s(jg_of * JPG + jo),
                                 start=jg_of == 0 and jo == 0,
                                 stop=stop_g and jo == JPG - 1)

        def combine_c(c):
            sfx = f"_{rg}_{c}"
            zr = tail_pool.tile([P, 1], F32, tag="zr", bufs=8,
                                name="zr" + sfx)
            nc.scalar.activation(zr[:], o1c[c][:, D:D + 1],
                                 AF.Identity, bias=epsc[:])
            gd = tail_pool.tile([P, 1], F32, tag="gd", bufs=8,
                                name="gd" + sfx)
            nc.scalar.mul(gd[:], o2c[c][:, D:D + 1], GAMMA / ONE9)
            tt_ = tail_pool.tile([P, D], F32, tag="t", bufs=4,
                                 name="t" + sfx)
            nc.scalar.mul(tt_[:], o1c[c][:, 0:D], gd[:])
            rz = tail_pool.tile([P, 1], F32, tag="rz", bufs=8,
                                name="rz" + sfx)
            nc.vector.reciprocal(rz[:], zr[:])
            ot = out_pool.tile([P, D], F16, tag="outt",
                               name="outt" + sfx)
            nc.vector.scalar_tensor_tensor(
                out=ot[:], in0=tt_[:], scalar=rz[:],
                in1=o2c[c][:, 0:D], op0=AOP.mult, op1=AOP.add)
            oeng = nc.sync if c % 2 == 0 else nc.scalar
            oeng.dma_start(
                out=out_d[rg * RB + c * P: rg * RB + (c + 1) * P, :],
                in_=ot[:])

        for jg in range(NG):
            pre = jg + 6
            if pre < NG:
                load_group(rg, pre)
            elif rg + 1 < KR:
                load_group(rg + 1, pre - NG)
            if jg == 0 and stash is not None:
                # elementwise + o2 were already emitted in rg-1's tail
                prev = (stash[0], 0)
                stash = None
                continue
            stats, ags = elementwise(rg, jg)
            first, last = jg == 0, jg == NG - 1
            if prev is not None:
                emit_o1(prev[0], prev[1], stop_g=False)
            for jo in range(JPG):
                for c in range(NC4):
                    nc.tensor.matmul(o2c[c][:, 0:D + 1],
                                     ags(jo)[:, c * P:(c + 1) * P],
                                     in2s(jg * JPG + jo),
                                     start=first and jo == 0,
                                     stop=last and jo == JPG - 1)
            prev = (stats, jg)

        # ---- tail. For interior rgs, interleave the o1 stop group, the
        # per-bank combine, and the NEXT rg's first o2 group at PE
        # emission level, so the scheduler anchors the boundary matmuls
        # to the per-bank combine chains. ----
        if rg + 1 < KR:
            nstats, nags = elementwise(rg + 1, 0)
            n_o2c = o1c  # role swap: next rg's o2 reuses this rg's o1 banks

            def o2n_c(c):
                for jo in range(JPG):
                    nc.tensor.matmul(n_o2c[c][:, 0:D + 1],
                                     nags(jo)[:, c * P:(c + 1) * P],
                                     in2s(jo), start=jo == 0, stop=False)

            emit_o1(prev[0], prev[1], stop_g=True, cs=[0, 1])
            with tc.high_priority():
                combine_c(0)
            o2n_c(0)
            emit_o1(prev[0], prev[1], stop_g=True, cs=[2])
            with tc.high_priority():
                combine_c(1)
            o2n_c(1)
            emit_o1(prev[0], prev[1], stop_g=True, cs=[3])
            with tc.high_priority():
                combine_c(2)
            o2n_c(2)
            with tc.high_priority():
                combine_c(3)
            o2n_c(3)
            stash = (nstats, nags)
        else:
            for c in range(NC4):
                emit_o1(prev[0], prev[1], stop_g=True, cs=[c])
                with tc.high_priority():
                    combine_c(c)


def build_nc(rows=N // N_CORES):
    nc = bacc.Bacc("TRN2", debug=False)
    with tile.TileContext(nc) as tc:
        with ExitStack() as ctx:
            build_kernel(nc, tc, ctx, rows)
    nc.compile()
    return nc


def kernel(input1, input2, adj, a1, a2, _trace=False):
    rows = input1.shape[0] // N_CORES
    KR, NG = rows // RB, JC // JPG
    nc = build_nc(rows=rows)

    e1 = (input1.astype(np.float64) @ a1.astype(np.float64)).ravel()
    e2 = (input2.astype(np.float64) @ a2.astype(np.float64)).ravel()

    vcol = np.stack([e2, np.exp(e2 - SHIFT), np.exp(0.2 * e2 - SHIFT)], 0)
    vcol = np.ascontiguousarray(
        vcol.astype(np.float32).reshape(3, JC, P).transpose(2, 0, 1))

    in2aug = np.ones((P, JC, D + 1), dtype=np.float32)
    in2aug[:, :, :D] = input2.reshape(JC, P, D).transpose(1, 0, 2)
    in2aug = np.ascontiguousarray((0.9 * in2aug).astype(ml_dtypes.bfloat16))

    in_maps = []
    for c in range(N_CORES):
        r0 = c * rows
        er = e1[r0:r0 + rows]
        u16 = np.stack([np.exp(er - SHIFT), np.exp(0.2 * er - SHIFT)], 0)
        u16 = u16.astype(ml_dtypes.bfloat16).reshape(2, KR, RB)
        u16 = u16.transpose(1, 0, 2)
        u16 = np.ascontiguousarray(np.broadcast_to(u16, (P, KR, 2, RB)))
        e1b = er.astype(ml_dtypes.bfloat16).reshape(KR, RB)
        e1b = np.ascontiguousarray(np.broadcast_to(e1b, (P, KR, RB)))
        ashard = adj[r0:r0 + rows].reshape(rows, JC, P)
        # [p, jc, r] -> [rg, g, p, jo, r]
        adjT = ashard.transpose(2, 1, 0)
        adjT = adjT.reshape(P, NG, JPG, KR, RB).transpose(3, 1, 0, 2, 4)
        adjA = np.ascontiguousarray(
            adjT[:, :, :, 0:4, :].astype(ml_dtypes.float8_e4m3))
        adjB = np.ascontiguousarray(
            adjT[:, :, :, 4:8, :].astype(ml_dtypes.bfloat16))
        in_maps.append({
            "adjA": adjA, "adjB": adjB,
            "in2aug": in2aug,
            "e1b": e1b,
            "u16": u16,
            "vcol": vcol,
        })

    res = run_bass_kernel_spmd(nc, in_maps, list(range(N_CORES)), trace=_trace)
    out = np.concatenate([res.results[c]["outs"] for c in range(N_CORES)],
                         axis=0).astype(np.float32)
    if _trace:
        return out, res
    return out
